# revision 2
# baseline (speedup 1.0000x reference)
"""Trainium2 Bass kernel for the CustomODELoss problem.

Full inputs:
    predicted_solution_batch [4096, 8192] f32
    target_solution_batch    [4096, 8192] f32
    c_input_batch            [4096]       f32
    x_eval_points            [8192]       f32   (uniform grid on [0, 1])

loss = mean((pred - target)^2)
     + mean((pred[r, idx_r] - 1)^2)
     + mean(((pred[r, idx_p] - pred[r, idx_m]) / ((idx_p - idx_m) * dx))^2)
where idx_r = argmin_j |x_j - c_r| (first index on ties).

Sharding: data-parallel over the batch dim, 512 rows per core on 8 cores.
Each core streams its pred/target slice once (memory-bound integral term)
and resolves the per-row grid index + finite-difference gather on device
via indirect DMA.  The index resolve is exact: a rounding-based candidate
j0 (always within 1 of the true argmin) is corrected by comparing the
f32 distances |x_j - c| of the 3 candidate grid points (via their
squares, which preserve order and ties) with the same first-index
tie-break as jnp.argmin.  A 5-wide pred window centered on j0 covers
every possible (idx-1, idx, idx+1) triple, so both indirect gathers
issue in parallel right after j0 is known.  The device emits
per-partition partial sums; the host sums the 8x128 partials and forms
the three means.

Stream schedule (from NTFF trace analysis): the load->subtract->release
loop has a ~3us latency that is independent of tile width, so narrow
tiles throttle the issue rate once the DMA queue drains.  All stream
tiles are therefore full-width (2048) except a short 1024+512+512 tail
that keeps the end-of-kernel serial chain (last load -> subtract ->
square -> reduce -> store) short.  pred loads issue from the SP HWDGE
queue (nc.sync) and targ loads from the ACT HWDGE queue (nc.scalar) so
the two in-order issue streams proceed in parallel and a buffer-release
wait on one never blocks the other.
"""

import numpy as np

import concourse.bacc as bacc
import concourse.bass as bass
import concourse.mybir as mybir
from concourse import tile
from concourse.bass_utils import run_bass_kernel_spmd

F32 = mybir.dt.float32
I32 = mybir.dt.int32
OP = mybir.AluOpType

B = 4096
N = 8192
NCORES = 8
BL = B // NCORES          # rows per core = 512
P = 128                   # SBUF partitions
RB = BL // P              # row groups per partition = 4
FT = 2048                 # free-dim tile for the streaming phase
PRE = 2                   # stream pairs emitted before phase-B part 1
SPLIT = 6                 # stream pairs emitted before phase-B part 2
W = 5                     # pred-window width

# Streaming tile schedule: (row_block, col_start, width) per pair.  Full
# width everywhere; only the final tile is split so the serial pipeline
# tail is short.
TILES = []
for _rb in range(BL // P):
    if _rb < BL // P - 1:
        TILES += [(_rb, _c * FT, FT) for _c in range(N // FT)]
    else:
        TILES += [(_rb, _c * FT, FT) for _c in range(N // FT - 1)]
        TILES += [(_rb, N - FT, FT // 2),
                  (_rb, N - FT // 2, FT // 4),
                  (_rb, N - FT // 4, FT // 4)]
NT = len(TILES)  # 18


def build_nc(debug=False):
    # Bacc (not plain Bass): its compile pipeline runs
    # generate_event_semaphores, which splits multi-sem waits into separate
    # event instructions — TRN2 allows at most 1 embedded wait per
    # instruction, and walrus codegen rejects the unsplit form.
    nc = bacc.Bacc()

    pred = nc.dram_tensor("pred", [BL, N], F32, kind="ExternalInput")
    targ = nc.dram_tensor("targ", [BL, N], F32, kind="ExternalInput")
    # c per core, reshaped host-side to [128, 4]: row r = p*RB + q
    cvec = nc.dram_tensor("cvec", [P, RB], F32, kind="ExternalInput")
    xev = nc.dram_tensor("xev", [N, 1], F32, kind="ExternalInput")
    dxb = nc.dram_tensor("dxb", [P, 1], F32, kind="ExternalInput")
    partials = nc.dram_tensor("partials", [P, 3], F32, kind="ExternalOutput")
    if debug:
        dbg = nc.dram_tensor("dbg", [P, 56], F32, kind="ExternalOutput")

    def view3(t):  # [128, 12] tile -> [128, 4, 3] AP
        return t[:].rearrange("p (q k) -> p q k", k=3)

    def view5(t):  # [128, 20] tile -> [128, 4, 5] AP
        return t[:].rearrange("p (q k) -> p q k", k=5)

    with tile.TileContext(nc) as tc:
        with (
            tc.tile_pool(name="ppool", bufs=10) as ppool,
            tc.tile_pool(name="tpool", bufs=10) as tpool,
            tc.tile_pool(name="dpool", bufs=4) as dpool,
            tc.tile_pool(name="pb", bufs=1) as pb,
        ):
            parts1 = pb.tile([P, NT], F32)
            pout = pb.tile([P, 3], F32)  # [t1_sum, t2_sum, t3_sum] columns

            def stream_pair(k):
                rb, cs, w = TILES[k]
                rs = rb * P
                pt = ppool.tile([P, FT], F32)
                tt = tpool.tile([P, FT], F32)
                nc.sync.dma_start(pt[:, :w], pred[rs:rs + P, cs:cs + w])
                nc.scalar.dma_start(tt[:, :w], targ[rs:rs + P, cs:cs + w])
                dt = dpool.tile([P, FT], F32)
                nc.vector.tensor_tensor(out=dt[:, :w], in0=pt[:, :w],
                                        in1=tt[:, :w], op=OP.subtract)
                # dt <- dt^2 in place; accum_out = row-sum
                nc.scalar.activation(
                    out=dt[:, :w], in_=dt[:, :w],
                    func=mybir.ActivationFunctionType.Square,
                    accum_out=parts1[:, k:k + 1],
                )

            # ========== phase A: first stream pairs go out first =========
            for k in range(PRE):
                stream_pair(k)

            # ========== phase B part 1: indices + both gathers ==========
            # The c -> j0 -> offsets chain is short, and the two indirect
            # gathers (SWDGE queue) fly while the streaming phase saturates
            # the HWDGE queues.  cvec/dxb load via nc.scalar so the sync
            # queue stays dedicated to pred stream loads.
            c_t = pb.tile([P, RB], F32)
            nc.scalar.dma_start(c_t[:], cvec[:, :])
            dx_t = pb.tile([P, 1], F32)
            nc.scalar.dma_start(dx_t[:], dxb[:, :])

            # j0 = int(c * (N-1)); any convert rounding mode keeps
            # |j0 - argmin| <= 1, which the 3-candidate check fixes.
            u = pb.tile([P, RB], F32)
            nc.vector.tensor_scalar(out=u[:], in0=c_t[:], scalar1=float(N - 1),
                                    scalar2=None, op0=OP.mult)
            j0i = pb.tile([P, RB], I32)
            nc.vector.tensor_copy(out=j0i[:], in_=u[:])
            j0f = pb.tile([P, RB], F32)
            nc.vector.tensor_copy(out=j0f[:], in_=j0i[:])
            jcc = pb.tile([P, RB], F32)
            nc.vector.tensor_scalar(out=jcc[:], in0=j0f[:], scalar1=1.0,
                                    scalar2=float(N - 2), op0=OP.max, op1=OP.min)

            # x window start: jc-1; candidate grid points {jc-1, jc, jc+1}
            s1f = pb.tile([P, RB], F32)
            nc.vector.tensor_scalar(out=s1f[:], in0=jcc[:], scalar1=1.0,
                                    scalar2=None, op0=OP.subtract)
            s1i = pb.tile([P, RB], I32)
            nc.vector.tensor_copy(out=s1i[:], in_=s1f[:])

            # pred window start: clip(j0-2, 0, N-W) — the 5-wide window
            # covers {jm, jstar, jp} for every jstar in {j0-1, j0, j0+1}.
            s5f = pb.tile([P, RB], F32)
            nc.vector.tensor_scalar(out=s5f[:], in0=j0f[:], scalar1=-2.0,
                                    scalar2=0.0, op0=OP.add, op1=OP.max)
            s5c = pb.tile([P, RB], F32)
            nc.vector.tensor_scalar(out=s5c[:], in0=s5f[:],
                                    scalar1=float(N - W), scalar2=None,
                                    op0=OP.min)
            s5i = pb.tile([P, RB], I32)
            nc.vector.tensor_copy(out=s5i[:], in_=s5c[:])
            rowbase = pb.tile([P, RB], I32)  # (p*RB + q) * N
            nc.gpsimd.iota(rowbase[:], pattern=[[N, RB]], base=0,
                           channel_multiplier=RB * N)
            offs = pb.tile([P, RB], I32)
            nc.vector.tensor_tensor(out=offs[:], in0=rowbase[:], in1=s5i[:],
                                    op=OP.add)

            # NOTE: hardware SWDGE honors only ONE offset per partition in an
            # indirect DMA (CoreSim accepts [128, RB] offsets, HW does not) —
            # issue one gather per row-group with [128, 1] offsets.
            xw = pb.tile([P, RB * 3], F32)
            for q in range(RB):
                nc.gpsimd.indirect_dma_start(
                    out=xw[:, 3 * q:3 * q + 3], out_offset=None, in_=xev[:, :],
                    in_offset=bass.IndirectOffsetOnAxis(
                        ap=s1i[:, q:q + 1], axis=0),
                )
            pw = pb.tile([P, RB * W], F32)
            for q in range(RB):
                nc.gpsimd.indirect_dma_start(
                    out=pw[:, W * q:W * q + W], out_offset=None,
                    in_=pred[:, :],
                    in_offset=bass.IndirectOffsetOnAxis(
                        ap=offs[:, q:q + 1], axis=1),
                )

            iota15 = pb.tile([P, RB * W], F32)
            nc.gpsimd.iota(iota15[:], pattern=[[0, RB], [1, W]], base=0,
                           channel_multiplier=0,
                           allow_small_or_imprecise_dtypes=True)

            # ========== phase A (continued) =============================
            for k in range(PRE, SPLIT):
                stream_pair(k)

            # ========== phase B part 2: select + finite difference ======
            # Pure DVE (no ACT hop): |d| comparisons use d*d — f32 squaring
            # is monotone in |d|, so order and ties match abs comparison.
            dsb = pb.tile([P, RB * 3], F32)
            nc.vector.tensor_tensor(out=view3(dsb), in0=view3(xw),
                                    in1=c_t[:].to_broadcast([P, RB, 3]),
                                    op=OP.subtract)
            dsq = pb.tile([P, RB * 3], F32)
            nc.vector.tensor_tensor(out=dsq[:], in0=dsb[:], in1=dsb[:],
                                    op=OP.mult)
            dm, d0, dp = dsq[:, 0::3], dsq[:, 1::3], dsq[:, 2::3]

            # first-argmin among {jc-1, jc, jc+1}:
            #   a = (dm<=d0)&(dm<=dp); b = (1-a)&(d0<=dp)
            #   jstar = jc + 1 - 2a - b
            t1b = pb.tile([P, RB], F32)
            nc.vector.tensor_tensor(out=t1b[:], in0=dm, in1=d0, op=OP.is_le)
            t2b = pb.tile([P, RB], F32)
            nc.vector.tensor_tensor(out=t2b[:], in0=dm, in1=dp, op=OP.is_le)
            a_t = pb.tile([P, RB], F32)
            nc.vector.tensor_tensor(out=a_t[:], in0=t1b[:], in1=t2b[:],
                                    op=OP.mult)
            t3b = pb.tile([P, RB], F32)
            nc.vector.tensor_tensor(out=t3b[:], in0=d0, in1=dp, op=OP.is_le)
            oma = pb.tile([P, RB], F32)
            nc.vector.tensor_scalar(out=oma[:], in0=a_t[:], scalar1=-1.0,
                                    scalar2=1.0, op0=OP.mult, op1=OP.add)
            b_t = pb.tile([P, RB], F32)
            nc.vector.tensor_tensor(out=b_t[:], in0=t3b[:], in1=oma[:],
                                    op=OP.mult)
            e1 = pb.tile([P, RB], F32)
            nc.vector.tensor_scalar(out=e1[:], in0=a_t[:], scalar1=-2.0,
                                    scalar2=1.0, op0=OP.mult, op1=OP.add)
            e2 = pb.tile([P, RB], F32)
            nc.vector.tensor_tensor(out=e2[:], in0=e1[:], in1=b_t[:],
                                    op=OP.subtract)
            jstar = pb.tile([P, RB], F32)
            nc.vector.tensor_tensor(out=jstar[:], in0=jcc[:], in1=e2[:],
                                    op=OP.add)

            # neighbors and in-window positions relative to s5
            jm = pb.tile([P, RB], F32)
            nc.vector.tensor_scalar(out=jm[:], in0=jstar[:], scalar1=-1.0,
                                    scalar2=0.0, op0=OP.add, op1=OP.max)
            jp = pb.tile([P, RB], F32)
            nc.vector.tensor_scalar(out=jp[:], in0=jstar[:], scalar1=1.0,
                                    scalar2=float(N - 1), op0=OP.add, op1=OP.min)
            p0 = pb.tile([P, RB], F32)
            nc.vector.tensor_tensor(out=p0[:], in0=jstar[:], in1=s5c[:],
                                    op=OP.subtract)
            pmp = pb.tile([P, RB], F32)
            nc.vector.tensor_tensor(out=pmp[:], in0=jm[:], in1=s5c[:],
                                    op=OP.subtract)
            ppp = pb.tile([P, RB], F32)
            nc.vector.tensor_tensor(out=ppp[:], in0=jp[:], in1=s5c[:],
                                    op=OP.subtract)

            # f(c): one-hot select of window position jstar
            m0 = pb.tile([P, RB * W], F32)
            nc.vector.tensor_tensor(out=view5(m0), in0=view5(iota15),
                                    in1=p0[:].to_broadcast([P, RB, W]),
                                    op=OP.is_equal)
            pr0 = pb.tile([P, RB * W], F32)
            nc.vector.tensor_tensor(out=pr0[:], in0=m0[:], in1=pw[:],
                                    op=OP.mult)
            fpc = pb.tile([P, RB], F32)
            nc.vector.reduce_sum(out=fpc[:], in_=view5(pr0),
                                 axis=mybir.AxisListType.X)

            # f'(c): (pred[jp] - pred[jm]) / ((jp-jm)*dx) via +/- one-hot
            mp_ = pb.tile([P, RB * W], F32)
            nc.vector.tensor_tensor(out=view5(mp_), in0=view5(iota15),
                                    in1=ppp[:].to_broadcast([P, RB, W]),
                                    op=OP.is_equal)
            mm_ = pb.tile([P, RB * W], F32)
            nc.vector.tensor_tensor(out=view5(mm_), in0=view5(iota15),
                                    in1=pmp[:].to_broadcast([P, RB, W]),
                                    op=OP.is_equal)
            wd = pb.tile([P, RB * W], F32)
            nc.vector.tensor_tensor(out=wd[:], in0=mp_[:], in1=mm_[:],
                                    op=OP.subtract)
            prd = pb.tile([P, RB * W], F32)
            nc.vector.tensor_tensor(out=prd[:], in0=wd[:], in1=pw[:],
                                    op=OP.mult)
            df = pb.tile([P, RB], F32)
            nc.vector.reduce_sum(out=df[:], in_=view5(prd),
                                 axis=mybir.AxisListType.X)
            qd = pb.tile([P, RB], F32)
            nc.vector.tensor_tensor(out=qd[:], in0=jp[:], in1=jm[:],
                                    op=OP.subtract)
            den = pb.tile([P, RB], F32)
            nc.vector.tensor_scalar(out=den[:], in0=qd[:], scalar1=dx_t[:, :1],
                                    scalar2=None, op0=OP.mult)
            rden = pb.tile([P, RB], F32)
            nc.vector.reciprocal(out=rden[:], in_=den[:])
            fpp = pb.tile([P, RB], F32)
            nc.vector.tensor_tensor(out=fpp[:], in0=df[:], in1=rden[:],
                                    op=OP.mult)

            # per-partition sums of (f(c)-1)^2 and f'(c)^2.
            # (tensor_tensor_reduce compiles but dies at runtime on HW —
            # use ACT Square with accumulate instead; these are terminal
            # outputs, so the ACT-stream position doesn't gate anything.)
            fpm1 = pb.tile([P, RB], F32)
            nc.vector.tensor_scalar(out=fpm1[:], in0=fpc[:], scalar1=-1.0,
                                    scalar2=None, op0=OP.add)
            sq2 = pb.tile([P, RB], F32)
            nc.scalar.activation(out=sq2[:], in_=fpm1[:],
                                 func=mybir.ActivationFunctionType.Square,
                                 accum_out=pout[:, 1:2])
            sq3 = pb.tile([P, RB], F32)
            nc.scalar.activation(out=sq3[:], in_=fpp[:],
                                 func=mybir.ActivationFunctionType.Square,
                                 accum_out=pout[:, 2:3])

            if debug:
                dbt = pb.tile([P, 56], F32)
                nc.vector.tensor_copy(out=dbt[:, 0:12], in_=xw[:])
                nc.vector.tensor_copy(out=dbt[:, 12:32], in_=pw[:])
                nc.vector.tensor_copy(out=dbt[:, 32:36], in_=jstar[:])
                nc.vector.tensor_copy(out=dbt[:, 36:40], in_=s5c[:])
                nc.vector.tensor_copy(out=dbt[:, 40:44], in_=fpc[:])
                nc.vector.tensor_copy(out=dbt[:, 44:48], in_=fpp[:])
                offf = pb.tile([P, RB], F32)
                nc.vector.tensor_copy(out=offf[:], in_=offs[:])
                nc.vector.tensor_copy(out=dbt[:, 48:52], in_=offf[:])
                nc.sync.dma_start(dbg[:, :], dbt[:])

            # ========== phase A (rest) ==================================
            for k in range(SPLIT, NT):
                stream_pair(k)

            nc.vector.reduce_sum(out=pout[:, 0:1], in_=parts1[:],
                                 axis=mybir.AxisListType.X)
            # single output store, on the scalar queue so it never sits
            # behind a pred stream load in the sync queue.
            nc.scalar.dma_start(partials[:, :], pout[:])

    return nc


_NC_CACHE = None


def _get_nc():
    global _NC_CACHE
    if _NC_CACHE is None:
        nc = build_nc()
        # Bacc runs its compile pipeline (register alloc, sync-wait
        # splitting) in finalize; the PJRT exec path requires it.
        nc.finalize()
        _NC_CACHE = nc
    return _NC_CACHE


def make_in_maps(predicted_solution_batch, target_solution_batch,
                 c_input_batch, x_eval_points):
    pred = np.ascontiguousarray(predicted_solution_batch, dtype=np.float32)
    targ = np.ascontiguousarray(target_solution_batch, dtype=np.float32)
    c = np.ascontiguousarray(c_input_batch, dtype=np.float32)
    x = np.ascontiguousarray(x_eval_points, dtype=np.float32)
    dx = np.float32(x[1]) - np.float32(x[0])
    dxb = np.full((P, 1), dx, dtype=np.float32)
    xev = x.reshape(N, 1)
    in_maps = []
    for i in range(NCORES):
        sl = slice(i * BL, (i + 1) * BL)
        in_maps.append({
            "pred": pred[sl],
            "targ": targ[sl],
            "cvec": c[sl].reshape(P, RB),
            "xev": xev,
            "dxb": dxb,
        })
    return in_maps


def reduce_partials(results):
    s = np.zeros(3, dtype=np.float64)
    for r in results:
        s += r["partials"].astype(np.float64).sum(axis=0)
    loss = s[0] / (B * N) + s[1] / B + s[2] / B
    return np.float32(loss)


def kernel(predicted_solution_batch, target_solution_batch,
           c_input_batch, x_eval_points):
    nc = _get_nc()
    in_maps = make_in_maps(predicted_solution_batch, target_solution_batch,
                           c_input_batch, x_eval_points)
    res = run_bass_kernel_spmd(nc, in_maps, core_ids=list(range(NCORES)))
    return reduce_partials(res.results)


# revision 4
# speedup vs baseline: 1.1287x; 1.1287x over previous
"""Trainium2 Bass kernel for the CustomODELoss problem.

Full inputs:
    predicted_solution_batch [4096, 8192] f32
    target_solution_batch    [4096, 8192] f32
    c_input_batch            [4096]       f32
    x_eval_points            [8192]       f32   (uniform grid on [0, 1])

loss = mean((pred - target)^2)
     + mean((pred[r, idx_r] - 1)^2)
     + mean(((pred[r, idx_p] - pred[r, idx_m]) / ((idx_p - idx_m) * dx))^2)
where idx_r = argmin_j |x_j - c_r| (first index on ties).

Sharding: data-parallel over the batch dim, 512 rows per core on 8 cores.
Each core streams its pred/target slice once (memory-bound integral term)
and resolves the per-row grid index + finite-difference gather on device
via indirect DMA.  The index resolve is exact: a rounding-based candidate
j0 (always within 1 of the true argmin) is corrected by comparing the
f32 distances |x_j - c| of the 3 candidate grid points (via their
squares, which preserve order and ties) with the same first-index
tie-break as jnp.argmin.  A 5-wide pred window centered on j0 covers
every possible (idx-1, idx, idx+1) triple, so both indirect gathers
issue in parallel right after j0 is known.  The device emits
per-partition partial sums; the host sums the 8x128 partials and forms
the three means.

Stream schedule (from NTFF trace analysis): the load->subtract->release
loop has a ~3us latency that is independent of tile width, so narrow
tiles throttle the issue rate once the DMA queue drains.  All stream
tiles are therefore full-width (2048) except a short 1024+512+512 tail
that keeps the end-of-kernel serial chain (last load -> subtract ->
square -> reduce -> store) short.  All stream loads issue from the SP
HWDGE queue (nc.sync), which carries nothing else; cvec/dxb loads and
the output store ride the ACT HWDGE queue.  (Putting targ loads on the
ACT queue was tried and regressed 13%: the queue is in-order, so each
ACTIVATE blocks the DMA issues behind it while waiting on its subtract.)
"""

import numpy as np

import concourse.bacc as bacc
import concourse.bass as bass
import concourse.mybir as mybir
from concourse import tile
from concourse.bass_utils import run_bass_kernel_spmd

F32 = mybir.dt.float32
I32 = mybir.dt.int32
OP = mybir.AluOpType

B = 4096
N = 8192
NCORES = 8
BL = B // NCORES          # rows per core = 512
P = 128                   # SBUF partitions
RB = BL // P              # row groups per partition = 4
FT = 2048                 # free-dim tile for the streaming phase
PRE = 2                   # stream pairs emitted before phase-B part 1
SPLIT = 6                 # stream pairs emitted before phase-B part 2
W = 5                     # pred-window width

# Streaming tile schedule: (row_block, col_start, width) per pair.  Full
# width everywhere; only the final tile is split so the serial pipeline
# tail is short.
TILES = []
for _rb in range(BL // P):
    if _rb < BL // P - 1:
        TILES += [(_rb, _c * FT, FT) for _c in range(N // FT)]
    else:
        TILES += [(_rb, _c * FT, FT) for _c in range(N // FT - 1)]
        TILES += [(_rb, N - FT, FT // 2),
                  (_rb, N - FT // 2, FT // 4),
                  (_rb, N - FT // 4, FT // 4)]
NT = len(TILES)  # 18


def build_nc(debug=False):
    # Bacc (not plain Bass): its compile pipeline runs
    # generate_event_semaphores, which splits multi-sem waits into separate
    # event instructions — TRN2 allows at most 1 embedded wait per
    # instruction, and walrus codegen rejects the unsplit form.
    nc = bacc.Bacc()

    pred = nc.dram_tensor("pred", [BL, N], F32, kind="ExternalInput")
    targ = nc.dram_tensor("targ", [BL, N], F32, kind="ExternalInput")
    # c per core, reshaped host-side to [128, 4]: row r = p*RB + q
    cvec = nc.dram_tensor("cvec", [P, RB], F32, kind="ExternalInput")
    xev = nc.dram_tensor("xev", [N, 1], F32, kind="ExternalInput")
    dxb = nc.dram_tensor("dxb", [P, 1], F32, kind="ExternalInput")
    partials = nc.dram_tensor("partials", [P, 3], F32, kind="ExternalOutput")
    if debug:
        dbg = nc.dram_tensor("dbg", [P, 56], F32, kind="ExternalOutput")

    def view3(t):  # [128, 12] tile -> [128, 4, 3] AP
        return t[:].rearrange("p (q k) -> p q k", k=3)

    def view5(t):  # [128, 20] tile -> [128, 4, 5] AP
        return t[:].rearrange("p (q k) -> p q k", k=5)

    with tile.TileContext(nc) as tc:
        with (
            tc.tile_pool(name="ppool", bufs=10) as ppool,
            tc.tile_pool(name="tpool", bufs=10) as tpool,
            tc.tile_pool(name="dpool", bufs=4) as dpool,
            tc.tile_pool(name="pb", bufs=1) as pb,
        ):
            parts1 = pb.tile([P, NT], F32)
            pout = pb.tile([P, 3], F32)  # [t1_sum, t2_sum, t3_sum] columns

            def stream_pair(k):
                rb, cs, w = TILES[k]
                rs = rb * P
                pt = ppool.tile([P, FT], F32)
                tt = tpool.tile([P, FT], F32)
                nc.sync.dma_start(pt[:, :w], pred[rs:rs + P, cs:cs + w])
                nc.sync.dma_start(tt[:, :w], targ[rs:rs + P, cs:cs + w])
                dt = dpool.tile([P, FT], F32)
                nc.vector.tensor_tensor(out=dt[:, :w], in0=pt[:, :w],
                                        in1=tt[:, :w], op=OP.subtract)
                # dt <- dt^2 in place; accum_out = row-sum
                nc.scalar.activation(
                    out=dt[:, :w], in_=dt[:, :w],
                    func=mybir.ActivationFunctionType.Square,
                    accum_out=parts1[:, k:k + 1],
                )

            # ========== phase A: first stream pairs go out first =========
            for k in range(PRE):
                stream_pair(k)

            # ========== phase B part 1: indices + both gathers ==========
            # The c -> j0 -> offsets chain is short, and the two indirect
            # gathers (SWDGE queue) fly while the streaming phase saturates
            # the HWDGE queues.  cvec/dxb load via nc.scalar so the sync
            # queue stays dedicated to pred stream loads.
            c_t = pb.tile([P, RB], F32)
            nc.scalar.dma_start(c_t[:], cvec[:, :])
            dx_t = pb.tile([P, 1], F32)
            nc.scalar.dma_start(dx_t[:], dxb[:, :])

            # j0 = int(c * (N-1)); any convert rounding mode keeps
            # |j0 - argmin| <= 1, which the 3-candidate check fixes.
            u = pb.tile([P, RB], F32)
            nc.vector.tensor_scalar(out=u[:], in0=c_t[:], scalar1=float(N - 1),
                                    scalar2=None, op0=OP.mult)
            j0i = pb.tile([P, RB], I32)
            nc.vector.tensor_copy(out=j0i[:], in_=u[:])
            j0f = pb.tile([P, RB], F32)
            nc.vector.tensor_copy(out=j0f[:], in_=j0i[:])
            jcc = pb.tile([P, RB], F32)
            nc.vector.tensor_scalar(out=jcc[:], in0=j0f[:], scalar1=1.0,
                                    scalar2=float(N - 2), op0=OP.max, op1=OP.min)

            # x window start: jc-1; candidate grid points {jc-1, jc, jc+1}
            s1f = pb.tile([P, RB], F32)
            nc.vector.tensor_scalar(out=s1f[:], in0=jcc[:], scalar1=1.0,
                                    scalar2=None, op0=OP.subtract)
            s1i = pb.tile([P, RB], I32)
            nc.vector.tensor_copy(out=s1i[:], in_=s1f[:])

            # pred window start: clip(j0-2, 0, N-W) — the 5-wide window
            # covers {jm, jstar, jp} for every jstar in {j0-1, j0, j0+1}.
            s5f = pb.tile([P, RB], F32)
            nc.vector.tensor_scalar(out=s5f[:], in0=j0f[:], scalar1=-2.0,
                                    scalar2=0.0, op0=OP.add, op1=OP.max)
            s5c = pb.tile([P, RB], F32)
            nc.vector.tensor_scalar(out=s5c[:], in0=s5f[:],
                                    scalar1=float(N - W), scalar2=None,
                                    op0=OP.min)
            s5i = pb.tile([P, RB], I32)
            nc.vector.tensor_copy(out=s5i[:], in_=s5c[:])
            rowbase = pb.tile([P, RB], I32)  # (p*RB + q) * N
            nc.gpsimd.iota(rowbase[:], pattern=[[N, RB]], base=0,
                           channel_multiplier=RB * N)
            offs = pb.tile([P, RB], I32)
            nc.vector.tensor_tensor(out=offs[:], in0=rowbase[:], in1=s5i[:],
                                    op=OP.add)

            # NOTE: hardware SWDGE honors only ONE offset per partition in an
            # indirect DMA (CoreSim accepts [128, RB] offsets, HW does not) —
            # issue one gather per row-group with [128, 1] offsets.
            xw = pb.tile([P, RB * 3], F32)
            for q in range(RB):
                nc.gpsimd.indirect_dma_start(
                    out=xw[:, 3 * q:3 * q + 3], out_offset=None, in_=xev[:, :],
                    in_offset=bass.IndirectOffsetOnAxis(
                        ap=s1i[:, q:q + 1], axis=0),
                )
            pw = pb.tile([P, RB * W], F32)
            for q in range(RB):
                nc.gpsimd.indirect_dma_start(
                    out=pw[:, W * q:W * q + W], out_offset=None,
                    in_=pred[:, :],
                    in_offset=bass.IndirectOffsetOnAxis(
                        ap=offs[:, q:q + 1], axis=1),
                )

            iota15 = pb.tile([P, RB * W], F32)
            nc.gpsimd.iota(iota15[:], pattern=[[0, RB], [1, W]], base=0,
                           channel_multiplier=0,
                           allow_small_or_imprecise_dtypes=True)

            # ========== phase A (continued) =============================
            for k in range(PRE, SPLIT):
                stream_pair(k)

            # ========== phase B part 2: select + finite difference ======
            # Pure DVE (no ACT hop): |d| comparisons use d*d — f32 squaring
            # is monotone in |d|, so order and ties match abs comparison.
            dsb = pb.tile([P, RB * 3], F32)
            nc.vector.tensor_tensor(out=view3(dsb), in0=view3(xw),
                                    in1=c_t[:].to_broadcast([P, RB, 3]),
                                    op=OP.subtract)
            dsq = pb.tile([P, RB * 3], F32)
            nc.vector.tensor_tensor(out=dsq[:], in0=dsb[:], in1=dsb[:],
                                    op=OP.mult)
            dm, d0, dp = dsq[:, 0::3], dsq[:, 1::3], dsq[:, 2::3]

            # first-argmin among {jc-1, jc, jc+1}:
            #   a = (dm<=d0)&(dm<=dp); b = (1-a)&(d0<=dp)
            #   jstar = jc + 1 - 2a - b
            t1b = pb.tile([P, RB], F32)
            nc.vector.tensor_tensor(out=t1b[:], in0=dm, in1=d0, op=OP.is_le)
            t2b = pb.tile([P, RB], F32)
            nc.vector.tensor_tensor(out=t2b[:], in0=dm, in1=dp, op=OP.is_le)
            a_t = pb.tile([P, RB], F32)
            nc.vector.tensor_tensor(out=a_t[:], in0=t1b[:], in1=t2b[:],
                                    op=OP.mult)
            t3b = pb.tile([P, RB], F32)
            nc.vector.tensor_tensor(out=t3b[:], in0=d0, in1=dp, op=OP.is_le)
            oma = pb.tile([P, RB], F32)
            nc.vector.tensor_scalar(out=oma[:], in0=a_t[:], scalar1=-1.0,
                                    scalar2=1.0, op0=OP.mult, op1=OP.add)
            b_t = pb.tile([P, RB], F32)
            nc.vector.tensor_tensor(out=b_t[:], in0=t3b[:], in1=oma[:],
                                    op=OP.mult)
            e1 = pb.tile([P, RB], F32)
            nc.vector.tensor_scalar(out=e1[:], in0=a_t[:], scalar1=-2.0,
                                    scalar2=1.0, op0=OP.mult, op1=OP.add)
            e2 = pb.tile([P, RB], F32)
            nc.vector.tensor_tensor(out=e2[:], in0=e1[:], in1=b_t[:],
                                    op=OP.subtract)
            jstar = pb.tile([P, RB], F32)
            nc.vector.tensor_tensor(out=jstar[:], in0=jcc[:], in1=e2[:],
                                    op=OP.add)

            # neighbors and in-window positions relative to s5
            jm = pb.tile([P, RB], F32)
            nc.vector.tensor_scalar(out=jm[:], in0=jstar[:], scalar1=-1.0,
                                    scalar2=0.0, op0=OP.add, op1=OP.max)
            jp = pb.tile([P, RB], F32)
            nc.vector.tensor_scalar(out=jp[:], in0=jstar[:], scalar1=1.0,
                                    scalar2=float(N - 1), op0=OP.add, op1=OP.min)
            p0 = pb.tile([P, RB], F32)
            nc.vector.tensor_tensor(out=p0[:], in0=jstar[:], in1=s5c[:],
                                    op=OP.subtract)
            pmp = pb.tile([P, RB], F32)
            nc.vector.tensor_tensor(out=pmp[:], in0=jm[:], in1=s5c[:],
                                    op=OP.subtract)
            ppp = pb.tile([P, RB], F32)
            nc.vector.tensor_tensor(out=ppp[:], in0=jp[:], in1=s5c[:],
                                    op=OP.subtract)

            # f(c): one-hot select of window position jstar
            m0 = pb.tile([P, RB * W], F32)
            nc.vector.tensor_tensor(out=view5(m0), in0=view5(iota15),
                                    in1=p0[:].to_broadcast([P, RB, W]),
                                    op=OP.is_equal)
            pr0 = pb.tile([P, RB * W], F32)
            nc.vector.tensor_tensor(out=pr0[:], in0=m0[:], in1=pw[:],
                                    op=OP.mult)
            fpc = pb.tile([P, RB], F32)
            nc.vector.reduce_sum(out=fpc[:], in_=view5(pr0),
                                 axis=mybir.AxisListType.X)

            # f'(c): (pred[jp] - pred[jm]) / ((jp-jm)*dx) via +/- one-hot
            mp_ = pb.tile([P, RB * W], F32)
            nc.vector.tensor_tensor(out=view5(mp_), in0=view5(iota15),
                                    in1=ppp[:].to_broadcast([P, RB, W]),
                                    op=OP.is_equal)
            mm_ = pb.tile([P, RB * W], F32)
            nc.vector.tensor_tensor(out=view5(mm_), in0=view5(iota15),
                                    in1=pmp[:].to_broadcast([P, RB, W]),
                                    op=OP.is_equal)
            wd = pb.tile([P, RB * W], F32)
            nc.vector.tensor_tensor(out=wd[:], in0=mp_[:], in1=mm_[:],
                                    op=OP.subtract)
            prd = pb.tile([P, RB * W], F32)
            nc.vector.tensor_tensor(out=prd[:], in0=wd[:], in1=pw[:],
                                    op=OP.mult)
            df = pb.tile([P, RB], F32)
            nc.vector.reduce_sum(out=df[:], in_=view5(prd),
                                 axis=mybir.AxisListType.X)
            qd = pb.tile([P, RB], F32)
            nc.vector.tensor_tensor(out=qd[:], in0=jp[:], in1=jm[:],
                                    op=OP.subtract)
            den = pb.tile([P, RB], F32)
            nc.vector.tensor_scalar(out=den[:], in0=qd[:], scalar1=dx_t[:, :1],
                                    scalar2=None, op0=OP.mult)
            rden = pb.tile([P, RB], F32)
            nc.vector.reciprocal(out=rden[:], in_=den[:])
            fpp = pb.tile([P, RB], F32)
            nc.vector.tensor_tensor(out=fpp[:], in0=df[:], in1=rden[:],
                                    op=OP.mult)

            # per-partition sums of (f(c)-1)^2 and f'(c)^2.
            # (tensor_tensor_reduce compiles but dies at runtime on HW —
            # use ACT Square with accumulate instead; these are terminal
            # outputs, so the ACT-stream position doesn't gate anything.)
            fpm1 = pb.tile([P, RB], F32)
            nc.vector.tensor_scalar(out=fpm1[:], in0=fpc[:], scalar1=-1.0,
                                    scalar2=None, op0=OP.add)
            sq2 = pb.tile([P, RB], F32)
            nc.scalar.activation(out=sq2[:], in_=fpm1[:],
                                 func=mybir.ActivationFunctionType.Square,
                                 accum_out=pout[:, 1:2])
            sq3 = pb.tile([P, RB], F32)
            nc.scalar.activation(out=sq3[:], in_=fpp[:],
                                 func=mybir.ActivationFunctionType.Square,
                                 accum_out=pout[:, 2:3])

            if debug:
                dbt = pb.tile([P, 56], F32)
                nc.vector.tensor_copy(out=dbt[:, 0:12], in_=xw[:])
                nc.vector.tensor_copy(out=dbt[:, 12:32], in_=pw[:])
                nc.vector.tensor_copy(out=dbt[:, 32:36], in_=jstar[:])
                nc.vector.tensor_copy(out=dbt[:, 36:40], in_=s5c[:])
                nc.vector.tensor_copy(out=dbt[:, 40:44], in_=fpc[:])
                nc.vector.tensor_copy(out=dbt[:, 44:48], in_=fpp[:])
                offf = pb.tile([P, RB], F32)
                nc.vector.tensor_copy(out=offf[:], in_=offs[:])
                nc.vector.tensor_copy(out=dbt[:, 48:52], in_=offf[:])
                nc.sync.dma_start(dbg[:, :], dbt[:])

            # ========== phase A (rest) ==================================
            for k in range(SPLIT, NT):
                stream_pair(k)

            nc.vector.reduce_sum(out=pout[:, 0:1], in_=parts1[:],
                                 axis=mybir.AxisListType.X)
            # single output store, on the scalar queue so it never sits
            # behind a pred stream load in the sync queue.
            nc.scalar.dma_start(partials[:, :], pout[:])

    return nc


_NC_CACHE = None


def _get_nc():
    global _NC_CACHE
    if _NC_CACHE is None:
        nc = build_nc()
        # Bacc runs its compile pipeline (register alloc, sync-wait
        # splitting) in finalize; the PJRT exec path requires it.
        nc.finalize()
        _NC_CACHE = nc
    return _NC_CACHE


def make_in_maps(predicted_solution_batch, target_solution_batch,
                 c_input_batch, x_eval_points):
    pred = np.ascontiguousarray(predicted_solution_batch, dtype=np.float32)
    targ = np.ascontiguousarray(target_solution_batch, dtype=np.float32)
    c = np.ascontiguousarray(c_input_batch, dtype=np.float32)
    x = np.ascontiguousarray(x_eval_points, dtype=np.float32)
    dx = np.float32(x[1]) - np.float32(x[0])
    dxb = np.full((P, 1), dx, dtype=np.float32)
    xev = x.reshape(N, 1)
    in_maps = []
    for i in range(NCORES):
        sl = slice(i * BL, (i + 1) * BL)
        in_maps.append({
            "pred": pred[sl],
            "targ": targ[sl],
            "cvec": c[sl].reshape(P, RB),
            "xev": xev,
            "dxb": dxb,
        })
    return in_maps


def reduce_partials(results):
    s = np.zeros(3, dtype=np.float64)
    for r in results:
        s += r["partials"].astype(np.float64).sum(axis=0)
    loss = s[0] / (B * N) + s[1] / B + s[2] / B
    return np.float32(loss)


def kernel(predicted_solution_batch, target_solution_batch,
           c_input_batch, x_eval_points):
    nc = _get_nc()
    in_maps = make_in_maps(predicted_solution_batch, target_solution_batch,
                           c_input_batch, x_eval_points)
    res = run_bass_kernel_spmd(nc, in_maps, core_ids=list(range(NCORES)))
    return reduce_partials(res.results)


# revision 6
# speedup vs baseline: 1.1462x; 1.0155x over previous
"""Trainium2 Bass kernel for the CustomODELoss problem.

Full inputs:
    predicted_solution_batch [4096, 8192] f32
    target_solution_batch    [4096, 8192] f32
    c_input_batch            [4096]       f32
    x_eval_points            [8192]       f32   (uniform grid on [0, 1])

loss = mean((pred - target)^2)
     + mean((pred[r, idx_r] - 1)^2)
     + mean(((pred[r, idx_p] - pred[r, idx_m]) / ((idx_p - idx_m) * dx))^2)
where idx_r = argmin_j |x_j - c_r| (first index on ties).

Sharding: data-parallel over the batch dim, 512 rows per core on 8 cores.
Each core streams its pred/target slice once (memory-bound integral term)
and resolves the per-row grid index + finite-difference gather on device
via indirect DMA.  The index resolve is exact: a rounding-based candidate
j0 (always within 1 of the true argmin) is corrected by comparing the
f32 distances |x_j - c| of the 3 candidate grid points (via their
squares, which preserve order and ties) with the same first-index
tie-break as jnp.argmin.  A 5-wide pred window centered on j0 covers
every possible (idx-1, idx, idx+1) triple, so both indirect gathers
issue in parallel right after j0 is known.  The device emits
per-partition partial sums; the host sums the 8x128 partials and forms
the three means.

Stream schedule (from NTFF trace analysis): the load->subtract->release
loop has a ~3us latency that is independent of tile width, so narrow
tiles throttle the issue rate once the DMA queue drains.  All stream
tiles are therefore full-width (2048) except a short 1024+512+512 tail
that keeps the end-of-kernel serial chain (last load -> subtract ->
square -> reduce -> store) short.  All stream loads issue from the SP
HWDGE queue (nc.sync), which carries nothing else; cvec/dxb loads and
the output store ride the ACT HWDGE queue.  (Putting targ loads on the
ACT queue was tried and regressed 13%: the queue is in-order, so each
ACTIVATE blocks the DMA issues behind it while waiting on its subtract.)
"""

import numpy as np

import concourse.bacc as bacc
import concourse.bass as bass
import concourse.mybir as mybir
from concourse import tile
from concourse.bass_utils import run_bass_kernel_spmd

F32 = mybir.dt.float32
I32 = mybir.dt.int32
OP = mybir.AluOpType

B = 4096
N = 8192
NCORES = 8
BL = B // NCORES          # rows per core = 512
P = 128                   # SBUF partitions
RB = BL // P              # row groups per partition = 4
FT = 4096                 # free-dim tile for the streaming phase
PRE = 2                   # stream pairs emitted before phase-B part 1
SPLIT = 3                 # stream pairs emitted before phase-B part 2
W = 5                     # pred-window width

# Streaming tile schedule: (row_block, col_start, width) per pair.  Wide
# tiles (16 KiB descriptors) maximize HBM efficiency and minimize the
# instruction count (every multi-wait instruction costs an event
# semaphore whose end-of-kernel reset is inside the measured window);
# the final tile is split 2048/1024/512/512 so the serial pipeline tail
# (last load -> subtract -> square -> reduce -> store) stays short.
TILES = []
for _rb in range(BL // P):
    if _rb < BL // P - 1:
        TILES += [(_rb, _c * FT, FT) for _c in range(N // FT)]
    else:
        TILES += [(_rb, 0, FT),
                  (_rb, 4096, 2048),
                  (_rb, 6144, 1024),
                  (_rb, 7168, 512),
                  (_rb, 7680, 512)]
NT = len(TILES)  # 11


def build_nc(debug=False):
    # Bacc (not plain Bass): its compile pipeline runs
    # generate_event_semaphores, which splits multi-sem waits into separate
    # event instructions — TRN2 allows at most 1 embedded wait per
    # instruction, and walrus codegen rejects the unsplit form.
    nc = bacc.Bacc()

    pred = nc.dram_tensor("pred", [BL, N], F32, kind="ExternalInput")
    targ = nc.dram_tensor("targ", [BL, N], F32, kind="ExternalInput")
    # c per core, reshaped host-side to [128, 4]: row r = p*RB + q
    cvec = nc.dram_tensor("cvec", [P, RB], F32, kind="ExternalInput")
    xev = nc.dram_tensor("xev", [N, 1], F32, kind="ExternalInput")
    dxb = nc.dram_tensor("dxb", [P, 1], F32, kind="ExternalInput")
    partials = nc.dram_tensor("partials", [P, 3], F32, kind="ExternalOutput")
    if debug:
        dbg = nc.dram_tensor("dbg", [P, 56], F32, kind="ExternalOutput")

    def view3(t):  # [128, 12] tile -> [128, 4, 3] AP
        return t[:].rearrange("p (q k) -> p q k", k=3)

    def view5(t):  # [128, 20] tile -> [128, 4, 5] AP
        return t[:].rearrange("p (q k) -> p q k", k=5)

    with tile.TileContext(nc) as tc:
        with (
            tc.tile_pool(name="ppool", bufs=5) as ppool,
            tc.tile_pool(name="tpool", bufs=5) as tpool,
            tc.tile_pool(name="dpool", bufs=2) as dpool,
            tc.tile_pool(name="pb", bufs=1) as pb,
        ):
            parts1 = pb.tile([P, NT], F32)
            pout = pb.tile([P, 3], F32)  # [t1_sum, t2_sum, t3_sum] columns

            def stream_pair(k):
                rb, cs, w = TILES[k]
                rs = rb * P
                pt = ppool.tile([P, FT], F32)
                tt = tpool.tile([P, FT], F32)
                nc.sync.dma_start(pt[:, :w], pred[rs:rs + P, cs:cs + w])
                nc.sync.dma_start(tt[:, :w], targ[rs:rs + P, cs:cs + w])
                dt = dpool.tile([P, FT], F32)
                nc.vector.tensor_tensor(out=dt[:, :w], in0=pt[:, :w],
                                        in1=tt[:, :w], op=OP.subtract)
                # dt <- dt^2 in place; accum_out = row-sum
                nc.scalar.activation(
                    out=dt[:, :w], in_=dt[:, :w],
                    func=mybir.ActivationFunctionType.Square,
                    accum_out=parts1[:, k:k + 1],
                )

            # ========== phase A: first stream pairs go out first =========
            for k in range(PRE):
                stream_pair(k)

            # ========== phase B part 1: indices + both gathers ==========
            # The c -> j0 -> offsets chain is short, and the two indirect
            # gathers (SWDGE queue) fly while the streaming phase saturates
            # the HWDGE queues.  cvec/dxb load via nc.scalar so the sync
            # queue stays dedicated to pred stream loads.
            c_t = pb.tile([P, RB], F32)
            nc.scalar.dma_start(c_t[:], cvec[:, :])
            dx_t = pb.tile([P, 1], F32)
            nc.scalar.dma_start(dx_t[:], dxb[:, :])

            # j0 = int(c * (N-1)); any convert rounding mode keeps
            # |j0 - argmin| <= 1, which the 3-candidate check fixes.
            u = pb.tile([P, RB], F32)
            nc.vector.tensor_scalar(out=u[:], in0=c_t[:], scalar1=float(N - 1),
                                    scalar2=None, op0=OP.mult)
            j0i = pb.tile([P, RB], I32)
            nc.vector.tensor_copy(out=j0i[:], in_=u[:])
            j0f = pb.tile([P, RB], F32)
            nc.vector.tensor_copy(out=j0f[:], in_=j0i[:])
            jcc = pb.tile([P, RB], F32)
            nc.vector.tensor_scalar(out=jcc[:], in0=j0f[:], scalar1=1.0,
                                    scalar2=float(N - 2), op0=OP.max, op1=OP.min)

            # x window start: jc-1; candidate grid points {jc-1, jc, jc+1}
            s1f = pb.tile([P, RB], F32)
            nc.vector.tensor_scalar(out=s1f[:], in0=jcc[:], scalar1=1.0,
                                    scalar2=None, op0=OP.subtract)
            s1i = pb.tile([P, RB], I32)
            nc.vector.tensor_copy(out=s1i[:], in_=s1f[:])

            # pred window start: clip(j0-2, 0, N-W) — the 5-wide window
            # covers {jm, jstar, jp} for every jstar in {j0-1, j0, j0+1}.
            s5f = pb.tile([P, RB], F32)
            nc.vector.tensor_scalar(out=s5f[:], in0=j0f[:], scalar1=-2.0,
                                    scalar2=0.0, op0=OP.add, op1=OP.max)
            s5c = pb.tile([P, RB], F32)
            nc.vector.tensor_scalar(out=s5c[:], in0=s5f[:],
                                    scalar1=float(N - W), scalar2=None,
                                    op0=OP.min)
            s5i = pb.tile([P, RB], I32)
            nc.vector.tensor_copy(out=s5i[:], in_=s5c[:])
            rowbase = pb.tile([P, RB], I32)  # (p*RB + q) * N
            nc.gpsimd.iota(rowbase[:], pattern=[[N, RB]], base=0,
                           channel_multiplier=RB * N)
            offs = pb.tile([P, RB], I32)
            nc.vector.tensor_tensor(out=offs[:], in0=rowbase[:], in1=s5i[:],
                                    op=OP.add)

            # NOTE: hardware SWDGE honors only ONE offset per partition in an
            # indirect DMA (CoreSim accepts [128, RB] offsets, HW does not) —
            # issue one gather per row-group with [128, 1] offsets.
            xw = pb.tile([P, RB * 3], F32)
            for q in range(RB):
                nc.gpsimd.indirect_dma_start(
                    out=xw[:, 3 * q:3 * q + 3], out_offset=None, in_=xev[:, :],
                    in_offset=bass.IndirectOffsetOnAxis(
                        ap=s1i[:, q:q + 1], axis=0),
                )
            pw = pb.tile([P, RB * W], F32)
            for q in range(RB):
                nc.gpsimd.indirect_dma_start(
                    out=pw[:, W * q:W * q + W], out_offset=None,
                    in_=pred[:, :],
                    in_offset=bass.IndirectOffsetOnAxis(
                        ap=offs[:, q:q + 1], axis=1),
                )

            iota15 = pb.tile([P, RB * W], F32)
            nc.gpsimd.iota(iota15[:], pattern=[[0, RB], [1, W]], base=0,
                           channel_multiplier=0,
                           allow_small_or_imprecise_dtypes=True)

            # ========== phase A (continued) =============================
            for k in range(PRE, SPLIT):
                stream_pair(k)

            # ========== phase B part 2: select + finite difference ======
            # Pure DVE (no ACT hop): |d| comparisons use d*d — f32 squaring
            # is monotone in |d|, so order and ties match abs comparison.
            dsb = pb.tile([P, RB * 3], F32)
            nc.vector.tensor_tensor(out=view3(dsb), in0=view3(xw),
                                    in1=c_t[:].to_broadcast([P, RB, 3]),
                                    op=OP.subtract)
            dsq = pb.tile([P, RB * 3], F32)
            nc.vector.tensor_tensor(out=dsq[:], in0=dsb[:], in1=dsb[:],
                                    op=OP.mult)
            dm, d0, dp = dsq[:, 0::3], dsq[:, 1::3], dsq[:, 2::3]

            # first-argmin among {jc-1, jc, jc+1}:
            #   a = (dm<=d0)&(dm<=dp); b = (1-a)&(d0<=dp)
            #   jstar = jc + 1 - 2a - b
            t1b = pb.tile([P, RB], F32)
            nc.vector.tensor_tensor(out=t1b[:], in0=dm, in1=d0, op=OP.is_le)
            t2b = pb.tile([P, RB], F32)
            nc.vector.tensor_tensor(out=t2b[:], in0=dm, in1=dp, op=OP.is_le)
            a_t = pb.tile([P, RB], F32)
            nc.vector.tensor_tensor(out=a_t[:], in0=t1b[:], in1=t2b[:],
                                    op=OP.mult)
            t3b = pb.tile([P, RB], F32)
            nc.vector.tensor_tensor(out=t3b[:], in0=d0, in1=dp, op=OP.is_le)
            oma = pb.tile([P, RB], F32)
            nc.vector.tensor_scalar(out=oma[:], in0=a_t[:], scalar1=-1.0,
                                    scalar2=1.0, op0=OP.mult, op1=OP.add)
            b_t = pb.tile([P, RB], F32)
            nc.vector.tensor_tensor(out=b_t[:], in0=t3b[:], in1=oma[:],
                                    op=OP.mult)
            e1 = pb.tile([P, RB], F32)
            nc.vector.tensor_scalar(out=e1[:], in0=a_t[:], scalar1=-2.0,
                                    scalar2=1.0, op0=OP.mult, op1=OP.add)
            e2 = pb.tile([P, RB], F32)
            nc.vector.tensor_tensor(out=e2[:], in0=e1[:], in1=b_t[:],
                                    op=OP.subtract)
            jstar = pb.tile([P, RB], F32)
            nc.vector.tensor_tensor(out=jstar[:], in0=jcc[:], in1=e2[:],
                                    op=OP.add)

            # neighbors and in-window positions relative to s5
            jm = pb.tile([P, RB], F32)
            nc.vector.tensor_scalar(out=jm[:], in0=jstar[:], scalar1=-1.0,
                                    scalar2=0.0, op0=OP.add, op1=OP.max)
            jp = pb.tile([P, RB], F32)
            nc.vector.tensor_scalar(out=jp[:], in0=jstar[:], scalar1=1.0,
                                    scalar2=float(N - 1), op0=OP.add, op1=OP.min)
            p0 = pb.tile([P, RB], F32)
            nc.vector.tensor_tensor(out=p0[:], in0=jstar[:], in1=s5c[:],
                                    op=OP.subtract)
            pmp = pb.tile([P, RB], F32)
            nc.vector.tensor_tensor(out=pmp[:], in0=jm[:], in1=s5c[:],
                                    op=OP.subtract)
            ppp = pb.tile([P, RB], F32)
            nc.vector.tensor_tensor(out=ppp[:], in0=jp[:], in1=s5c[:],
                                    op=OP.subtract)

            # f(c): one-hot select of window position jstar
            m0 = pb.tile([P, RB * W], F32)
            nc.vector.tensor_tensor(out=view5(m0), in0=view5(iota15),
                                    in1=p0[:].to_broadcast([P, RB, W]),
                                    op=OP.is_equal)
            pr0 = pb.tile([P, RB * W], F32)
            nc.vector.tensor_tensor(out=pr0[:], in0=m0[:], in1=pw[:],
                                    op=OP.mult)
            fpc = pb.tile([P, RB], F32)
            nc.vector.reduce_sum(out=fpc[:], in_=view5(pr0),
                                 axis=mybir.AxisListType.X)

            # f'(c): (pred[jp] - pred[jm]) / ((jp-jm)*dx) via +/- one-hot
            mp_ = pb.tile([P, RB * W], F32)
            nc.vector.tensor_tensor(out=view5(mp_), in0=view5(iota15),
                                    in1=ppp[:].to_broadcast([P, RB, W]),
                                    op=OP.is_equal)
            mm_ = pb.tile([P, RB * W], F32)
            nc.vector.tensor_tensor(out=view5(mm_), in0=view5(iota15),
                                    in1=pmp[:].to_broadcast([P, RB, W]),
                                    op=OP.is_equal)
            wd = pb.tile([P, RB * W], F32)
            nc.vector.tensor_tensor(out=wd[:], in0=mp_[:], in1=mm_[:],
                                    op=OP.subtract)
            prd = pb.tile([P, RB * W], F32)
            nc.vector.tensor_tensor(out=prd[:], in0=wd[:], in1=pw[:],
                                    op=OP.mult)
            df = pb.tile([P, RB], F32)
            nc.vector.reduce_sum(out=df[:], in_=view5(prd),
                                 axis=mybir.AxisListType.X)
            qd = pb.tile([P, RB], F32)
            nc.vector.tensor_tensor(out=qd[:], in0=jp[:], in1=jm[:],
                                    op=OP.subtract)
            den = pb.tile([P, RB], F32)
            nc.vector.tensor_scalar(out=den[:], in0=qd[:], scalar1=dx_t[:, :1],
                                    scalar2=None, op0=OP.mult)
            rden = pb.tile([P, RB], F32)
            nc.vector.reciprocal(out=rden[:], in_=den[:])
            fpp = pb.tile([P, RB], F32)
            nc.vector.tensor_tensor(out=fpp[:], in0=df[:], in1=rden[:],
                                    op=OP.mult)

            # per-partition sums of (f(c)-1)^2 and f'(c)^2.
            # (tensor_tensor_reduce compiles but dies at runtime on HW —
            # use ACT Square with accumulate instead; these are terminal
            # outputs, so the ACT-stream position doesn't gate anything.)
            fpm1 = pb.tile([P, RB], F32)
            nc.vector.tensor_scalar(out=fpm1[:], in0=fpc[:], scalar1=-1.0,
                                    scalar2=None, op0=OP.add)
            sq2 = pb.tile([P, RB], F32)
            nc.scalar.activation(out=sq2[:], in_=fpm1[:],
                                 func=mybir.ActivationFunctionType.Square,
                                 accum_out=pout[:, 1:2])
            sq3 = pb.tile([P, RB], F32)
            nc.scalar.activation(out=sq3[:], in_=fpp[:],
                                 func=mybir.ActivationFunctionType.Square,
                                 accum_out=pout[:, 2:3])

            if debug:
                dbt = pb.tile([P, 56], F32)
                nc.vector.tensor_copy(out=dbt[:, 0:12], in_=xw[:])
                nc.vector.tensor_copy(out=dbt[:, 12:32], in_=pw[:])
                nc.vector.tensor_copy(out=dbt[:, 32:36], in_=jstar[:])
                nc.vector.tensor_copy(out=dbt[:, 36:40], in_=s5c[:])
                nc.vector.tensor_copy(out=dbt[:, 40:44], in_=fpc[:])
                nc.vector.tensor_copy(out=dbt[:, 44:48], in_=fpp[:])
                offf = pb.tile([P, RB], F32)
                nc.vector.tensor_copy(out=offf[:], in_=offs[:])
                nc.vector.tensor_copy(out=dbt[:, 48:52], in_=offf[:])
                nc.sync.dma_start(dbg[:, :], dbt[:])

            # ========== phase A (rest) ==================================
            for k in range(SPLIT, NT):
                stream_pair(k)

            nc.vector.reduce_sum(out=pout[:, 0:1], in_=parts1[:],
                                 axis=mybir.AxisListType.X)
            # single output store, on the scalar queue so it never sits
            # behind a pred stream load in the sync queue.
            nc.scalar.dma_start(partials[:, :], pout[:])

    return nc


_NC_CACHE = None


def _get_nc():
    global _NC_CACHE
    if _NC_CACHE is None:
        nc = build_nc()
        # Bacc runs its compile pipeline (register alloc, sync-wait
        # splitting) in finalize; the PJRT exec path requires it.
        nc.finalize()
        _NC_CACHE = nc
    return _NC_CACHE


def make_in_maps(predicted_solution_batch, target_solution_batch,
                 c_input_batch, x_eval_points):
    pred = np.ascontiguousarray(predicted_solution_batch, dtype=np.float32)
    targ = np.ascontiguousarray(target_solution_batch, dtype=np.float32)
    c = np.ascontiguousarray(c_input_batch, dtype=np.float32)
    x = np.ascontiguousarray(x_eval_points, dtype=np.float32)
    dx = np.float32(x[1]) - np.float32(x[0])
    dxb = np.full((P, 1), dx, dtype=np.float32)
    xev = x.reshape(N, 1)
    in_maps = []
    for i in range(NCORES):
        sl = slice(i * BL, (i + 1) * BL)
        in_maps.append({
            "pred": pred[sl],
            "targ": targ[sl],
            "cvec": c[sl].reshape(P, RB),
            "xev": xev,
            "dxb": dxb,
        })
    return in_maps


def reduce_partials(results):
    s = np.zeros(3, dtype=np.float64)
    for r in results:
        s += r["partials"].astype(np.float64).sum(axis=0)
    loss = s[0] / (B * N) + s[1] / B + s[2] / B
    return np.float32(loss)


def kernel(predicted_solution_batch, target_solution_batch,
           c_input_batch, x_eval_points):
    nc = _get_nc()
    in_maps = make_in_maps(predicted_solution_batch, target_solution_batch,
                           c_input_batch, x_eval_points)
    res = run_bass_kernel_spmd(nc, in_maps, core_ids=list(range(NCORES)))
    return reduce_partials(res.results)
